# revision 5
# baseline (speedup 1.0000x reference)
"""Trainium2 Bass kernel for BatchGroupItN (iterative whitening group norm).

Math (reference):
    x: (N=64, C=256, H=56, W=56) fp32.  Group of channel c is g = c % 32.
    xg[g, m] collects all elements with c % 32 == g  (m = 512*3136 per group).
    sigma = cov(xg) + eps*I  (32x32); wm = sigma^{-1/2} via 5 Newton-Schulz
    iters on trace-normalized sigma; out = (wm @ (xg - mu)) scattered back,
    then * weight + bias.

Strategy (8 cores, data-parallel over batch N):
    Each core owns 8 batches = 16 contiguous slabs of [128 channels, 3136 hw].
    Channel partition p of a slab belongs to group p % 32.
    Pass 1: per slab, PE-transpose [128,128] chunks -> T [m,c] tiles; Gram
    matmuls accumulate S128 = sum T^T T in PSUM ([128,128]; its four 32x32
    diagonal blocks sum to the raw second-moment matrix S = sum x x^T).
    Channel sums come from an in-place ACT copy with accum_out.
    Fold S128/sums to 32-wide via selector matmuls, AllReduce a packed
    [32,64] buffer across the 8 cores, then every core runs the (tiny)
    Newton-Schulz iterations and builds a block-diagonal WM = diag(wm x4).
    Pass 2: y = WM @ x per [128,512] chunk on the PE, then one ACT affine
    (scale=weight, bias=bias - wm@mu * weight) and DMA out.  The first
    RESIDENT slabs stay in SBUF between passes; the rest are re-read.
"""

import numpy as np

import concourse.bass as bass
import concourse.bacc as bacc
import concourse.tile as tile
from concourse import bass_utils, mybir

F32 = mybir.dt.float32
AX = mybir.AxisListType
OP = mybir.AluOpType
AF = mybir.ActivationFunctionType

N_CORES = 8
G = 32
T_ITERS = 5
EPS = 1e-5
N, C, H, W = 64, 256, 56, 56
HW = H * W  # 3136
P = 128
SLABS = 16  # per core: 8 batches x 2 channel-halves of 128
M_TOTAL = float(N * (C // G) * HW)  # 1,605,632 elements per group
RESIDENT = 10  # slabs kept in SBUF between pass 1 and pass 2
GRPS = (HW + 511) // 512  # 7: six full 512 groups + one 64 tail


def _emit(ctx, tc, x, w2, b2, i128, bd, out):
    nc = tc.nc

    consts = ctx.enter_context(tc.tile_pool(name="consts", bufs=1))
    single = ctx.enter_context(tc.tile_pool(name="single", bufs=1))
    ns = ctx.enter_context(tc.tile_pool(name="ns", bufs=3))
    xres = ctx.enter_context(tc.tile_pool(name="xres", bufs=RESIDENT))
    xstream = ctx.enter_context(tc.tile_pool(name="xstream", bufs=2))
    tp = ctx.enter_context(tc.tile_pool(name="tp", bufs=3))
    outp = ctx.enter_context(tc.tile_pool(name="outp", bufs=2))
    psA = ctx.enter_context(tc.tile_pool(name="psA", bufs=1, space="PSUM"))
    psT = ctx.enter_context(tc.tile_pool(name="psT", bufs=2, space="PSUM"))
    psY = ctx.enter_context(tc.tile_pool(name="psY", bufs=2, space="PSUM"))
    psS = ctx.enter_context(tc.tile_pool(name="psS", bufs=2, space="PSUM"))
    dram = ctx.enter_context(tc.tile_pool(name="dram", bufs=1, space="DRAM"))

    I128 = consts.tile([P, P], F32)
    nc.sync.dma_start(I128, i128)
    BD = consts.tile([P, P], F32)
    nc.sync.dma_start(BD, bd)
    I32 = I128[0:G, 0:G]
    ones = consts.tile([P, G], F32)
    nc.vector.memset(ones, 1.0)
    wsb = consts.tile([P, 2], F32)
    bsb = consts.tile([P, 2], F32)
    for h in range(2):
        nc.sync.dma_start(wsb[:, h : h + 1], w2[h])
        nc.sync.dma_start(bsb[:, h : h + 1], b2[h])

    # ---------------- pass 1: statistics ----------------
    psum_S = psA.tile([P, P], F32, tag="pS")
    sums_parts = single.tile([P, SLABS], F32)

    xt_tiles = [None] * SLABS
    n_grams = SLABS * 25  # 6 groups x 4 chunks + 1 tail chunk, per slab
    gram_i = 0
    for s in range(SLABS):
        if s < RESIDENT:
            xt = xres.tile([P, HW], F32, tag="xr")
            xt_tiles[s] = xt
        else:
            xt = xstream.tile([P, HW], F32, tag="xs")
        nc.sync.dma_start(xt, x[s])
        # channel sums for this slab: in-place identity copy, accum_out = row sum
        nc.scalar.activation(
            out=xt, in_=xt, func=AF.Identity, accum_out=sums_parts[:, s : s + 1]
        )
        for grp in range(GRPS):
            off = 512 * grp
            wd = min(512, HW - off)  # 512 or 64
            nch = (wd + 127) // 128  # 4 or 1
            pt = psT.tile([P, 512], F32, tag="pt")
            for k in range(nch):
                cw = min(128, wd - 128 * k)  # 128 or 64
                nc.tensor.transpose(
                    pt[0:cw, 128 * k : 128 * k + P],
                    xt[:, off + 128 * k : off + 128 * k + cw],
                    I128,
                )
            tsb = tp.tile([P, 512], F32, tag="tsb")
            if wd == 512:
                nc.vector.tensor_copy(tsb, pt)
            else:
                nc.vector.tensor_copy(tsb[0:wd, 0:P], pt[0:wd, 0:P])
            for k in range(nch):
                cw = min(128, wd - 128 * k)
                lhs = tsb[0:cw, 128 * k : 128 * k + P]
                gram_i += 1
                nc.tensor.matmul(
                    psum_S,
                    lhsT=lhs,
                    rhs=lhs,
                    start=(gram_i == 1),
                    stop=(gram_i == n_grams),
                )

    # ---------------- fold + all-reduce ----------------
    sums128 = single.tile([P, 1], F32)
    nc.vector.tensor_reduce(sums128, sums_parts, AX.X, OP.add)
    Ssb = single.tile([P, P], F32)
    nc.vector.tensor_copy(Ssb, psum_S)
    ps32 = psS.tile([G, 64], F32, tag="sps")
    for i in range(4):
        nc.tensor.matmul(
            ps32[:, 0:G],
            lhsT=BD[:, G * i : G * i + G],
            rhs=Ssb[:, G * i : G * i + G],
            start=(i == 0),
            stop=(i == 3),
        )
    nc.tensor.matmul(ps32[:, G : G + 1], lhsT=BD[:, 0:G], rhs=sums128, start=True, stop=True)
    pack = single.tile([G, 64], F32)
    nc.vector.memset(pack, 0.0)
    nc.vector.tensor_copy(pack[:, 0 : G + 1], ps32[:, 0 : G + 1])

    cc_in = dram.tile([G, 64], F32)
    cc_out = dram.tile([G, 64], F32)
    nc.gpsimd.dma_start(cc_in, pack)
    nc.gpsimd.collective_compute(
        "AllReduce",
        OP.add,
        replica_groups=[list(range(N_CORES))],
        ins=[cc_in.opt()],
        outs=[cc_out.opt()],
    )
    packr = single.tile([G, 64], F32)
    nc.gpsimd.dma_start(packr, cc_out)

    # ---------------- sigma, trace, Newton-Schulz ----------------
    inv_m = 1.0 / M_TOTAL
    mu = single.tile([G, 1], F32)
    nc.vector.tensor_scalar_mul(mu, packr[:, G : G + 1], inv_m)
    ps_mr = psS.tile([1, G], F32, tag="sps")
    nc.tensor.transpose(ps_mr, mu, I32)
    murow = single.tile([1, G], F32)
    nc.vector.tensor_copy(murow, ps_mr)
    ps_mm = psS.tile([G, G], F32, tag="sps")
    nc.tensor.matmul(ps_mm, lhsT=murow, rhs=murow, start=True, stop=True)
    sigma = single.tile([G, G], F32)
    nc.vector.tensor_scalar_mul(sigma, packr[:, 0:G], inv_m)
    nc.vector.tensor_sub(sigma, sigma, ps_mm)
    epsI = single.tile([G, G], F32)
    nc.vector.tensor_scalar_mul(epsI, I32, EPS)
    nc.vector.tensor_add(sigma, sigma, epsI)

    diag = single.tile([G, G], F32)
    nc.vector.tensor_mul(diag, sigma, I32)
    dcol = single.tile([G, 1], F32)
    nc.vector.tensor_reduce(dcol, diag, AX.X, OP.add)
    ps_tr = psS.tile([1, 1], F32, tag="sps")
    nc.tensor.matmul(ps_tr, lhsT=dcol, rhs=ones[0:G, 0:1], start=True, stop=True)
    trsb = single.tile([1, 1], F32)
    nc.vector.tensor_copy(trsb, ps_tr)
    tinv = single.tile([1, 1], F32)
    nc.vector.reciprocal(tinv, trsb)
    ps_b32 = psS.tile([G, 1], F32, tag="sps")
    nc.tensor.matmul(ps_b32, lhsT=ones[0:1, 0:G], rhs=tinv, start=True, stop=True)
    tinv32 = single.tile([G, 1], F32)
    nc.vector.tensor_copy(tinv32, ps_b32)
    sigN = single.tile([G, G], F32)
    nc.vector.tensor_scalar_mul(sigN, sigma, tinv32)
    stinv = single.tile([G, 1], F32)
    nc.scalar.sqrt(stinv, tinv32)

    Pcur = single.tile([G, G], F32, tag="P0")
    nc.vector.tensor_copy(Pcur, I32)
    for _ in range(T_ITERS):
        psa = psS.tile([G, G], F32, tag="sps")
        nc.tensor.matmul(psa, lhsT=Pcur, rhs=Pcur, start=True, stop=True)
        asb = ns.tile([G, G], F32, tag="nsA")
        nc.vector.tensor_copy(asb, psa)
        psb_ = psS.tile([G, G], F32, tag="sps")
        nc.tensor.matmul(psb_, lhsT=asb, rhs=Pcur, start=True, stop=True)
        bsb_ = ns.tile([G, G], F32, tag="nsB")
        nc.vector.tensor_copy(bsb_, psb_)
        psc = psS.tile([G, G], F32, tag="sps")
        nc.tensor.matmul(psc, lhsT=bsb_, rhs=sigN, start=True, stop=True)
        chalf = ns.tile([G, G], F32, tag="nsC")
        nc.vector.tensor_scalar_mul(chalf, psc, 0.5)
        pn = ns.tile([G, G], F32, tag="nsP")
        nc.vector.tensor_scalar_mul(pn, Pcur, 1.5)
        nc.vector.tensor_sub(pn, pn, chalf)
        Pcur = pn

    wm = single.tile([G, G], F32)
    nc.vector.tensor_scalar_mul(wm, Pcur, stinv)

    # block-diagonal WM = diag(wm, wm, wm, wm)
    ps_wm = psA.tile([P, P], F32, tag="pWM")
    for i in range(4):
        nc.tensor.matmul(
            ps_wm[G * i : G * i + G, G * i : G * i + G],
            lhsT=wm,
            rhs=I32,
            start=True,
            stop=True,
            tile_position=(0, G * i),
        )
    WM = single.tile([P, P], F32)
    nc.vector.memset(WM, 0.0)
    for i in range(4):
        nc.vector.tensor_copy(
            WM[G * i : G * i + G, G * i : G * i + G],
            ps_wm[G * i : G * i + G, G * i : G * i + G],
        )

    # per-partition affine: scale = weight, bias = bias - (wm @ mu) * weight
    ps_v = psS.tile([G, 1], F32, tag="sps")
    nc.tensor.matmul(ps_v, lhsT=wm, rhs=mu, start=True, stop=True)
    vsb = single.tile([G, 1], F32)
    nc.vector.tensor_copy(vsb, ps_v)
    ps_v128 = psS.tile([P, 1], F32, tag="sps")
    nc.tensor.matmul(ps_v128, lhsT=BD[0:G, :], rhs=vsb, start=True, stop=True)
    v128 = single.tile([P, 1], F32)
    nc.vector.tensor_copy(v128, ps_v128)
    badj = single.tile([P, 2], F32)
    for h in range(2):
        nc.vector.tensor_mul(badj[:, h : h + 1], v128, wsb[:, h : h + 1])
        nc.vector.tensor_sub(badj[:, h : h + 1], bsb[:, h : h + 1], badj[:, h : h + 1])

    # ---------------- pass 2: normalize ----------------
    for s in range(SLABS):
        if s < RESIDENT:
            xt = xt_tiles[s]
        else:
            xt = xstream.tile([P, HW], F32, tag="xs")
            nc.sync.dma_start(xt, x[s])
        h = s % 2
        osb = outp.tile([P, HW], F32, tag="osb")
        for grp in range(GRPS):
            off = 512 * grp
            wd = min(512, HW - off)
            py = psY.tile([P, 512], F32, tag="py")
            nc.tensor.matmul(
                py[:, 0:wd], lhsT=WM, rhs=xt[:, off : off + wd], start=True, stop=True
            )
            nc.scalar.activation(
                out=osb[:, off : off + wd],
                in_=py[:, 0:wd],
                func=AF.Identity,
                bias=badj[:, h : h + 1],
                scale=wsb[:, h : h + 1],
            )
        nc.sync.dma_start(out[s], osb)


def _fix_gram_flags():
    pass  # start/stop handled inline


_BUILT = None


def _build():
    global _BUILT
    if _BUILT is not None:
        return _BUILT
    nc = bacc.Bacc(
        "TRN2",
        target_bir_lowering=False,
        debug=False,
        enable_asserts=False,
        num_devices=N_CORES,
    )
    x_d = nc.dram_tensor("x", [SLABS, P, HW], F32, kind="ExternalInput")
    w_d = nc.dram_tensor("w2", [2, P, 1], F32, kind="ExternalInput")
    b_d = nc.dram_tensor("b2", [2, P, 1], F32, kind="ExternalInput")
    i_d = nc.dram_tensor("i128", [P, P], F32, kind="ExternalInput")
    bd_d = nc.dram_tensor("bd128", [P, P], F32, kind="ExternalInput")
    o_d = nc.dram_tensor("out", [SLABS, P, HW], F32, kind="ExternalOutput")
    from contextlib import ExitStack

    with tile.TileContext(nc) as tc, ExitStack() as ctx:
        _emit(ctx, tc, x_d.ap(), w_d.ap(), b_d.ap(), i_d.ap(), bd_d.ap(), o_d.ap())
    nc.compile()
    _BUILT = nc
    return nc


def kernel(x, weight, bias, trace=False):
    x = np.ascontiguousarray(np.asarray(x, dtype=np.float32))
    weight = np.asarray(weight, dtype=np.float32)
    bias = np.asarray(bias, dtype=np.float32)
    assert x.shape == (N, C, H, W)

    nc = _build()

    w2 = np.ascontiguousarray(weight.reshape(2, P, 1))
    b2 = np.ascontiguousarray(bias.reshape(2, P, 1))
    i128 = np.eye(P, dtype=np.float32)
    idx = np.arange(P)
    bd128 = (idx[:, None] % G == idx[None, :] % G).astype(np.float32)

    xs = x.reshape(N_CORES, SLABS, P, HW)
    in_maps = [
        {"x": xs[c], "w2": w2, "b2": b2, "i128": i128, "bd128": bd128}
        for c in range(N_CORES)
    ]
    res = bass_utils.run_bass_kernel_spmd(
        nc, in_maps, core_ids=list(range(N_CORES)), trace=trace
    )
    out = np.concatenate(
        [r["out"].reshape(1, N // N_CORES, C, H, W) for r in res.results], axis=0
    ).reshape(N, C, H, W)
    if trace:
        return out, res
    return out


# revision 8
# speedup vs baseline: 19848.1786x; 19848.1786x over previous
"""Trainium2 Bass kernel for BatchGroupItN (iterative whitening group norm).

Math (reference):
    x: (N=64, C=256, H=56, W=56) fp32.  Group of channel c is g = c % 32.
    xg[g, m] collects all elements with c % 32 == g  (m = 512*3136 per group).
    sigma = cov(xg) + eps*I  (32x32); wm = sigma^{-1/2} via 5 Newton-Schulz
    iters on trace-normalized sigma; out = (wm @ (xg - mu)) scattered back,
    then * weight + bias.

Strategy (8 cores, data-parallel over batch N):
    Each core owns 8 batches = 16 contiguous slabs of [128 channels, 3136 hw].
    Channel partition p of a slab belongs to group p % 32.
    Pass 1: per slab, PE-transpose [128,128] chunks -> T [m,c] tiles; Gram
    matmuls accumulate S128 = sum T^T T in PSUM ([128,128]; its four 32x32
    diagonal blocks sum to the raw second-moment matrix S = sum x x^T).
    Channel sums come from an in-place ACT copy with accum_out.
    Fold S128/sums to 32-wide via selector matmuls, AllReduce a packed
    [32,64] buffer across the 8 cores, then every core runs the (tiny)
    Newton-Schulz iterations and builds a block-diagonal WM = diag(wm x4).
    Pass 2: y = WM @ x per [128,512] chunk on the PE, then one ACT affine
    (scale=weight, bias=bias - wm@mu * weight) and DMA out.  The first
    RESIDENT slabs stay in SBUF between passes; the rest are re-read.
"""

import numpy as np

import concourse.bass as bass
import concourse.bacc as bacc
import concourse.tile as tile
from concourse import bass_utils, mybir

F32 = mybir.dt.float32
AX = mybir.AxisListType
OP = mybir.AluOpType
AF = mybir.ActivationFunctionType

N_CORES = 8
G = 32
T_ITERS = 5
EPS = 1e-5
N, C, H, W = 64, 256, 56, 56
HW = H * W  # 3136
P = 128
SLABS = 16  # per core: 8 batches x 2 channel-halves of 128
M_TOTAL = float(N * (C // G) * HW)  # 1,605,632 elements per group
RESIDENT = 10  # slabs kept in SBUF between pass 1 and pass 2
GRPS = (HW + 511) // 512  # 7: six full 512 groups + one 64 tail


def _emit(ctx, tc, x, w2, b2, i128, bd, out):
    nc = tc.nc

    consts = ctx.enter_context(tc.tile_pool(name="consts", bufs=1))
    single = ctx.enter_context(tc.tile_pool(name="single", bufs=1))
    ns = ctx.enter_context(tc.tile_pool(name="ns", bufs=3))
    xres = ctx.enter_context(tc.tile_pool(name="xres", bufs=RESIDENT))
    xstream = ctx.enter_context(tc.tile_pool(name="xstream", bufs=2))
    tp = ctx.enter_context(tc.tile_pool(name="tp", bufs=3))
    outp = ctx.enter_context(tc.tile_pool(name="outp", bufs=2))
    psA = ctx.enter_context(tc.tile_pool(name="psA", bufs=1, space="PSUM"))
    psT = ctx.enter_context(tc.tile_pool(name="psT", bufs=2, space="PSUM"))
    psY = ctx.enter_context(tc.tile_pool(name="psY", bufs=2, space="PSUM"))
    psS = ctx.enter_context(tc.tile_pool(name="psS", bufs=2, space="PSUM"))
    dram = ctx.enter_context(tc.tile_pool(name="dram", bufs=1, space="DRAM"))

    I128 = consts.tile([P, P], F32)
    nc.sync.dma_start(I128, i128)
    BD = consts.tile([P, P], F32)
    nc.sync.dma_start(BD, bd)
    I32 = I128[0:G, 0:G]
    ones = consts.tile([P, G], F32)
    nc.vector.memset(ones, 1.0)
    wsb = consts.tile([P, 2], F32)
    bsb = consts.tile([P, 2], F32)
    for h in range(2):
        nc.sync.dma_start(wsb[:, h : h + 1], w2[h])
        nc.sync.dma_start(bsb[:, h : h + 1], b2[h])

    # ---------------- pass 1: statistics ----------------
    psum_S = psA.tile([P, P], F32, tag="pS")
    sums_parts = single.tile([P, SLABS], F32)

    xt_tiles = [None] * SLABS
    n_grams = SLABS * 25  # 6 groups x 4 chunks + 1 tail chunk, per slab
    gram_i = 0
    for s in range(SLABS):
        if s < RESIDENT:
            xt = xres.tile([P, HW], F32, tag="xr")
            xt_tiles[s] = xt
        else:
            xt = xstream.tile([P, HW], F32, tag="xs")
        nc.sync.dma_start(xt, x[s])
        # channel sums for this slab: in-place identity copy, accum_out = row sum
        nc.scalar.activation(
            out=xt, in_=xt, func=AF.Identity, accum_out=sums_parts[:, s : s + 1]
        )
        for grp in range(GRPS):
            off = 512 * grp
            wd = min(512, HW - off)  # 512 or 64
            nch = (wd + 127) // 128  # 4 or 1
            pt = psT.tile([P, 512], F32, tag="pt")
            for k in range(nch):
                cw = min(128, wd - 128 * k)  # 128 or 64
                nc.tensor.transpose(
                    pt[0:cw, 128 * k : 128 * k + P],
                    xt[:, off + 128 * k : off + 128 * k + cw],
                    I128,
                )
            tsb = tp.tile([P, 512], F32, tag="tsb")
            if wd == 512:
                nc.vector.tensor_copy(tsb, pt)
            else:
                nc.vector.tensor_copy(tsb[0:wd, 0:P], pt[0:wd, 0:P])
            for k in range(nch):
                cw = min(128, wd - 128 * k)
                lhs = tsb[0:cw, 128 * k : 128 * k + P]
                gram_i += 1
                nc.tensor.matmul(
                    psum_S,
                    lhsT=lhs,
                    rhs=lhs,
                    start=(gram_i == 1),
                    stop=(gram_i == n_grams),
                )

    # ---------------- fold + all-reduce ----------------
    sums128 = single.tile([P, 1], F32)
    nc.vector.tensor_reduce(sums128, sums_parts, AX.X, OP.add)
    Ssb = single.tile([P, P], F32)
    nc.vector.tensor_copy(Ssb, psum_S)
    ps32 = psS.tile([G, 64], F32, tag="sps")
    for i in range(4):
        # lhsT = columns of I128: selects ONLY row-block i (BD would sum all
        # four row-blocks and contaminate with cross-block covariances)
        nc.tensor.matmul(
            ps32[:, 0:G],
            lhsT=I128[:, G * i : G * i + G],
            rhs=Ssb[:, G * i : G * i + G],
            start=(i == 0),
            stop=(i == 3),
        )
    nc.tensor.matmul(ps32[:, G : G + 1], lhsT=BD[:, 0:G], rhs=sums128, start=True, stop=True)
    pack = single.tile([G, 64], F32)
    nc.vector.memset(pack, 0.0)
    nc.vector.tensor_copy(pack[:, 0 : G + 1], ps32[:, 0 : G + 1])

    cc_in = dram.tile([G, 64], F32)
    cc_out = dram.tile([G, 64], F32)
    nc.gpsimd.dma_start(cc_in, pack)
    nc.gpsimd.collective_compute(
        "AllReduce",
        OP.add,
        replica_groups=[list(range(N_CORES))],
        ins=[cc_in.opt()],
        outs=[cc_out.opt()],
    )
    packr = single.tile([G, 64], F32)
    nc.gpsimd.dma_start(packr, cc_out)

    # ---------------- sigma, trace, Newton-Schulz ----------------
    inv_m = 1.0 / M_TOTAL
    mu = single.tile([G, 1], F32)
    nc.vector.tensor_scalar_mul(mu, packr[:, G : G + 1], inv_m)
    ps_mr = psS.tile([1, G], F32, tag="sps")
    nc.tensor.transpose(ps_mr, mu, I32)
    murow = single.tile([1, G], F32)
    nc.vector.tensor_copy(murow, ps_mr)
    ps_mm = psS.tile([G, G], F32, tag="sps")
    nc.tensor.matmul(ps_mm, lhsT=murow, rhs=murow, start=True, stop=True)
    sigma = single.tile([G, G], F32)
    nc.vector.tensor_scalar_mul(sigma, packr[:, 0:G], inv_m)
    nc.vector.tensor_sub(sigma, sigma, ps_mm)
    epsI = single.tile([G, G], F32)
    nc.vector.tensor_scalar_mul(epsI, I32, EPS)
    nc.vector.tensor_add(sigma, sigma, epsI)

    diag = single.tile([G, G], F32)
    nc.vector.tensor_mul(diag, sigma, I32)
    dcol = single.tile([G, 1], F32)
    nc.vector.tensor_reduce(dcol, diag, AX.X, OP.add)
    ps_tr = psS.tile([1, 1], F32, tag="sps")
    nc.tensor.matmul(ps_tr, lhsT=dcol, rhs=ones[0:G, 0:1], start=True, stop=True)
    trsb = single.tile([1, 1], F32)
    nc.vector.tensor_copy(trsb, ps_tr)
    tinv = single.tile([1, 1], F32)
    nc.vector.reciprocal(tinv, trsb)
    ps_b32 = psS.tile([G, 1], F32, tag="sps")
    nc.tensor.matmul(ps_b32, lhsT=ones[0:1, 0:G], rhs=tinv, start=True, stop=True)
    tinv32 = single.tile([G, 1], F32)
    nc.vector.tensor_copy(tinv32, ps_b32)
    sigN = single.tile([G, G], F32)
    nc.vector.tensor_scalar_mul(sigN, sigma, tinv32)
    stinv = single.tile([G, 1], F32)
    nc.scalar.sqrt(stinv, tinv32)

    Pcur = single.tile([G, G], F32, tag="P0")
    nc.vector.tensor_copy(Pcur, I32)
    for _ in range(T_ITERS):
        psa = psS.tile([G, G], F32, tag="sps")
        nc.tensor.matmul(psa, lhsT=Pcur, rhs=Pcur, start=True, stop=True)
        asb = ns.tile([G, G], F32, tag="nsA")
        nc.vector.tensor_copy(asb, psa)
        psb_ = psS.tile([G, G], F32, tag="sps")
        nc.tensor.matmul(psb_, lhsT=asb, rhs=Pcur, start=True, stop=True)
        bsb_ = ns.tile([G, G], F32, tag="nsB")
        nc.vector.tensor_copy(bsb_, psb_)
        psc = psS.tile([G, G], F32, tag="sps")
        nc.tensor.matmul(psc, lhsT=bsb_, rhs=sigN, start=True, stop=True)
        chalf = ns.tile([G, G], F32, tag="nsC")
        nc.vector.tensor_scalar_mul(chalf, psc, 0.5)
        pn = ns.tile([G, G], F32, tag="nsP")
        nc.vector.tensor_scalar_mul(pn, Pcur, 1.5)
        nc.vector.tensor_sub(pn, pn, chalf)
        Pcur = pn

    wm = single.tile([G, G], F32)
    nc.vector.tensor_scalar_mul(wm, Pcur, stinv)

    # block-diagonal WM = diag(wm, wm, wm, wm)
    ps_wm = psA.tile([P, P], F32, tag="pWM")
    for i in range(4):
        nc.tensor.matmul(
            ps_wm[G * i : G * i + G, G * i : G * i + G],
            lhsT=wm,
            rhs=I32,
            start=True,
            stop=True,
            tile_position=(0, G * i),
        )
    WM = single.tile([P, P], F32)
    nc.vector.memset(WM, 0.0)
    for i in range(4):
        nc.vector.tensor_copy(
            WM[G * i : G * i + G, G * i : G * i + G],
            ps_wm[G * i : G * i + G, G * i : G * i + G],
        )

    # per-partition affine: scale = weight, bias = bias - (wm @ mu) * weight
    ps_v = psS.tile([G, 1], F32, tag="sps")
    nc.tensor.matmul(ps_v, lhsT=wm, rhs=mu, start=True, stop=True)
    vsb = single.tile([G, 1], F32)
    nc.vector.tensor_copy(vsb, ps_v)
    ps_v128 = psS.tile([P, 1], F32, tag="sps")
    nc.tensor.matmul(ps_v128, lhsT=BD[0:G, :], rhs=vsb, start=True, stop=True)
    v128 = single.tile([P, 1], F32)
    nc.vector.tensor_copy(v128, ps_v128)
    badj = single.tile([P, 2], F32)
    for h in range(2):
        nc.vector.tensor_mul(badj[:, h : h + 1], v128, wsb[:, h : h + 1])
        nc.vector.tensor_sub(badj[:, h : h + 1], bsb[:, h : h + 1], badj[:, h : h + 1])

    # ---------------- pass 2: normalize ----------------
    for s in range(SLABS):
        if s < RESIDENT:
            xt = xt_tiles[s]
        else:
            xt = xstream.tile([P, HW], F32, tag="xs")
            nc.sync.dma_start(xt, x[s])
        h = s % 2
        osb = outp.tile([P, HW], F32, tag="osb")
        for grp in range(GRPS):
            off = 512 * grp
            wd = min(512, HW - off)
            py = psY.tile([P, 512], F32, tag="py")
            nc.tensor.matmul(
                py[:, 0:wd], lhsT=WM, rhs=xt[:, off : off + wd], start=True, stop=True
            )
            nc.scalar.activation(
                out=osb[:, off : off + wd],
                in_=py[:, 0:wd],
                func=AF.Identity,
                bias=badj[:, h : h + 1],
                scale=wsb[:, h : h + 1],
            )
        nc.sync.dma_start(out[s], osb)


def _fix_gram_flags():
    pass  # start/stop handled inline


_BUILT = None


def _build():
    global _BUILT
    if _BUILT is not None:
        return _BUILT
    nc = bacc.Bacc(
        "TRN2",
        target_bir_lowering=False,
        debug=False,
        enable_asserts=False,
        num_devices=N_CORES,
    )
    x_d = nc.dram_tensor("x", [SLABS, P, HW], F32, kind="ExternalInput")
    w_d = nc.dram_tensor("w2", [2, P, 1], F32, kind="ExternalInput")
    b_d = nc.dram_tensor("b2", [2, P, 1], F32, kind="ExternalInput")
    i_d = nc.dram_tensor("i128", [P, P], F32, kind="ExternalInput")
    bd_d = nc.dram_tensor("bd128", [P, P], F32, kind="ExternalInput")
    o_d = nc.dram_tensor("out", [SLABS, P, HW], F32, kind="ExternalOutput")
    from contextlib import ExitStack

    with tile.TileContext(nc) as tc, ExitStack() as ctx:
        _emit(ctx, tc, x_d.ap(), w_d.ap(), b_d.ap(), i_d.ap(), bd_d.ap(), o_d.ap())
    nc.compile()
    _BUILT = nc
    return nc


def kernel(x, weight, bias, trace=False, tmpdir=None):
    x = np.ascontiguousarray(np.asarray(x, dtype=np.float32))
    weight = np.asarray(weight, dtype=np.float32)
    bias = np.asarray(bias, dtype=np.float32)
    assert x.shape == (N, C, H, W)

    nc = _build()

    w2 = np.ascontiguousarray(weight.reshape(2, P, 1))
    b2 = np.ascontiguousarray(bias.reshape(2, P, 1))
    i128 = np.eye(P, dtype=np.float32)
    idx = np.arange(P)
    bd128 = (idx[:, None] % G == idx[None, :] % G).astype(np.float32)

    xs = x.reshape(N_CORES, SLABS, P, HW)
    in_maps = [
        {"x": xs[c], "w2": w2, "b2": b2, "i128": i128, "bd128": bd128}
        for c in range(N_CORES)
    ]
    res = bass_utils.run_bass_kernel_spmd(
        nc, in_maps, core_ids=list(range(N_CORES)), trace=trace, tmpdir=tmpdir
    )
    out = np.concatenate(
        [r["out"].reshape(1, N // N_CORES, C, H, W) for r in res.results], axis=0
    ).reshape(N, C, H, W)
    if trace:
        return out, res
    return out


# revision 10
# speedup vs baseline: 25857.0716x; 1.3027x over previous
"""Trainium2 Bass kernel for BatchGroupItN (iterative whitening group norm).

Math (reference):
    x: (N=64, C=256, H=56, W=56) fp32.  Group of channel c is g = c % 32.
    xg[g, m] collects all elements with c % 32 == g  (m = 512*3136 per group).
    sigma = cov(xg) + eps*I  (32x32); wm = sigma^{-1/2} via 5 Newton-Schulz
    iters on trace-normalized sigma; out = (wm @ (xg - mu)) scattered back,
    then * weight + bias.

Strategy (8 cores, data-parallel over batch N):
    Each core owns 8 batches = 16 contiguous slabs of [128 channels, 3136 hw].
    Channel partition p of a slab belongs to group p % 32.
    Pass 1 (bf16): cast each slab to bf16, PE-transpose [128,128] chunks ->
    T [m,c] tiles, Gram matmuls accumulate S128 = sum T^T T in PSUM; the four
    32x32 diagonal blocks of S128 sum to S = sum x x^T.  Channel sums come
    from an in-place fp32 ACT copy with accum_out (full precision).
    Fold S128/sums to 32-wide via selector matmuls, AllReduce a packed
    [32,64] buffer across the 8 cores, then every core runs the (tiny)
    Newton-Schulz iterations and builds a block-diagonal WM = diag(wm x4).
    Pass 2 (fp32): y = WM @ x per [128,512] chunk on the PE, then one
    per-partition affine (scale=weight, bias=bias - wm@mu * weight,
    alternating DVE/ACT) and DMA out.  The first RESIDENT slabs stay in
    SBUF between passes; the rest are re-read from HBM.
"""

import numpy as np

import concourse.bass as bass
import concourse.bacc as bacc
import concourse.tile as tile
from concourse import bass_utils, mybir

F32 = mybir.dt.float32
BF16 = mybir.dt.bfloat16
AX = mybir.AxisListType
OP = mybir.AluOpType
AF = mybir.ActivationFunctionType

N_CORES = 8
G = 32
T_ITERS = 5
EPS = 1e-5
N, C, H, W = 64, 256, 56, 56
HW = H * W  # 3136
P = 128
SLABS = 16  # per core: 8 batches x 2 channel-halves of 128
M_TOTAL = float(N * (C // G) * HW)  # 1,605,632 elements per group
RESIDENT = 10  # slabs kept in SBUF between pass 1 and pass 2
GRPS = (HW + 511) // 512  # 7: six full 512 groups + one 64 tail
N_WARM = 24  # dummy matmuls keeping the PE warm through the all-reduce


def _emit(ctx, tc, x, w2, b2, i128, bd, out):
    nc = tc.nc

    consts = ctx.enter_context(tc.tile_pool(name="consts", bufs=1))
    single = ctx.enter_context(tc.tile_pool(name="single", bufs=1))
    ns = ctx.enter_context(tc.tile_pool(name="ns", bufs=3))
    xres = ctx.enter_context(tc.tile_pool(name="xres", bufs=RESIDENT))
    xstream = ctx.enter_context(tc.tile_pool(name="xstream", bufs=2))
    xbp = ctx.enter_context(tc.tile_pool(name="xbp", bufs=2))
    tp = ctx.enter_context(tc.tile_pool(name="tp", bufs=3))
    outp = ctx.enter_context(tc.tile_pool(name="outp", bufs=2))
    psA = ctx.enter_context(tc.tile_pool(name="psA", bufs=1, space="PSUM"))
    psT = ctx.enter_context(tc.tile_pool(name="psT", bufs=2, space="PSUM"))
    psY = ctx.enter_context(tc.tile_pool(name="psY", bufs=4, space="PSUM"))
    psS = ctx.enter_context(tc.tile_pool(name="psS", bufs=1, space="PSUM"))
    dram = ctx.enter_context(tc.tile_pool(name="dram", bufs=1, space="DRAM"))

    I128 = consts.tile([P, P], F32)
    nc.sync.dma_start(I128, i128)
    I128b = consts.tile([P, P], BF16)
    nc.vector.tensor_copy(I128b, I128)
    BD = consts.tile([P, P], F32)
    nc.sync.dma_start(BD, bd)
    I32 = I128[0:G, 0:G]
    ones = consts.tile([P, G], F32)
    nc.vector.memset(ones, 1.0)
    wsb = consts.tile([P, 2], F32)
    bsb = consts.tile([P, 2], F32)
    for h in range(2):
        nc.sync.dma_start(wsb[:, h : h + 1], w2[h])
        nc.sync.dma_start(bsb[:, h : h + 1], b2[h])

    # ---------------- pass 1: statistics (bf16 compute, fp32 sums) ---------
    psum_S = psA.tile([P, P], F32, tag="pS")
    sums_parts = single.tile([P, SLABS], F32)

    xt_tiles = [None] * SLABS
    n_grams = SLABS * 25  # 6 groups x 4 chunks + 1 tail chunk, per slab
    gram_i = 0
    for s in range(SLABS):
        if s < RESIDENT:
            xt = xres.tile([P, HW], F32, tag="xr")
            xt_tiles[s] = xt
        else:
            xt = xstream.tile([P, HW], F32, tag="xs")
        nc.sync.dma_start(xt, x[s])
        xb = xbp.tile([P, HW], BF16, tag="xb")
        nc.vector.tensor_copy(xb, xt)  # fp32 -> bf16 cast
        # channel sums (full fp32): in-place identity copy, accum_out = row sum
        nc.scalar.activation(
            out=xt, in_=xt, func=AF.Identity, accum_out=sums_parts[:, s : s + 1]
        )
        for grp in range(GRPS):
            off = 512 * grp
            wd = min(512, HW - off)  # 512 or 64
            nch = (wd + 127) // 128  # 4 or 1
            pt = psT.tile([P, 512], BF16, tag="pt")
            for k in range(nch):
                cw = min(128, wd - 128 * k)  # 128 or 64
                nc.tensor.transpose(
                    pt[0:cw, 128 * k : 128 * k + P],
                    xb[:, off + 128 * k : off + 128 * k + cw],
                    I128b,
                )
            tsb = tp.tile([P, 512], BF16, tag="tsb")
            if wd == 512:
                nc.vector.tensor_copy(tsb, pt)
            else:
                nc.vector.tensor_copy(tsb[0:wd, 0:P], pt[0:wd, 0:P])
            for k in range(nch):
                cw = min(128, wd - 128 * k)
                lhs = tsb[0:cw, 128 * k : 128 * k + P]
                gram_i += 1
                nc.tensor.matmul(
                    psum_S,
                    lhsT=lhs,
                    rhs=lhs,
                    start=(gram_i == 1),
                    stop=(gram_i == n_grams),
                )

    # ---------------- fold + all-reduce ----------------
    sums128 = single.tile([P, 1], F32)
    nc.vector.tensor_reduce(sums128, sums_parts, AX.X, OP.add)
    Ssb = single.tile([P, P], F32)
    nc.vector.tensor_copy(Ssb, psum_S)
    ps32 = psS.tile([G, 64], F32, tag="sps")
    for i in range(4):
        # lhsT = columns of I128: selects ONLY row-block i of S128
        nc.tensor.matmul(
            ps32[:, 0:G],
            lhsT=I128[:, G * i : G * i + G],
            rhs=Ssb[:, G * i : G * i + G],
            start=(i == 0),
            stop=(i == 3),
        )
    nc.tensor.matmul(ps32[:, G : G + 1], lhsT=BD[:, 0:G], rhs=sums128, start=True, stop=True)
    pack = single.tile([G, 64], F32)
    nc.vector.memset(pack, 0.0)
    nc.vector.tensor_copy(pack[:, 0 : G + 1], ps32[:, 0 : G + 1])

    cc_in = dram.tile([G, 64], F32)
    cc_out = dram.tile([G, 64], F32)
    nc.gpsimd.dma_start(cc_in, pack)
    nc.gpsimd.collective_compute(
        "AllReduce",
        OP.add,
        replica_groups=[list(range(N_CORES))],
        ins=[cc_in.opt()],
        outs=[cc_out.opt()],
    )

    # keep the PE's HAM clock warm through the all-reduce wait: identity
    # matmuls on already-resident data into otherwise-idle psum banks
    warm_src = xt_tiles[0]
    for wi in range(N_WARM):
        pw = psY.tile([P, 512], F32, tag="py")
        nc.tensor.matmul(
            pw, lhsT=I128, rhs=warm_src[:, 0:512], start=True, stop=True
        )

    packr = single.tile([G, 64], F32)
    nc.gpsimd.dma_start(packr, cc_out)

    # ---------------- sigma, trace, Newton-Schulz ----------------
    inv_m = 1.0 / M_TOTAL
    mu = single.tile([G, 1], F32)
    nc.vector.tensor_scalar_mul(mu, packr[:, G : G + 1], inv_m)
    ps_mr = psS.tile([1, G], F32, tag="sps")
    nc.tensor.transpose(ps_mr, mu, I32)
    murow = single.tile([1, G], F32)
    nc.vector.tensor_copy(murow, ps_mr)
    ps_mm = psS.tile([G, G], F32, tag="sps")
    nc.tensor.matmul(ps_mm, lhsT=murow, rhs=murow, start=True, stop=True)
    sigma = single.tile([G, G], F32)
    nc.vector.tensor_scalar_mul(sigma, packr[:, 0:G], inv_m)
    nc.vector.tensor_sub(sigma, sigma, ps_mm)
    epsI = single.tile([G, G], F32)
    nc.vector.tensor_scalar_mul(epsI, I32, EPS)
    nc.vector.tensor_add(sigma, sigma, epsI)

    diag = single.tile([G, G], F32)
    nc.vector.tensor_mul(diag, sigma, I32)
    dcol = single.tile([G, 1], F32)
    nc.vector.tensor_reduce(dcol, diag, AX.X, OP.add)
    ps_tr = psS.tile([1, 1], F32, tag="sps")
    nc.tensor.matmul(ps_tr, lhsT=dcol, rhs=ones[0:G, 0:1], start=True, stop=True)
    trsb = single.tile([1, 1], F32)
    nc.vector.tensor_copy(trsb, ps_tr)
    tinv = single.tile([1, 1], F32)
    nc.vector.reciprocal(tinv, trsb)
    ps_b32 = psS.tile([G, 1], F32, tag="sps")
    nc.tensor.matmul(ps_b32, lhsT=ones[0:1, 0:G], rhs=tinv, start=True, stop=True)
    tinv32 = single.tile([G, 1], F32)
    nc.vector.tensor_copy(tinv32, ps_b32)
    sigN = single.tile([G, G], F32)
    nc.vector.tensor_scalar_mul(sigN, sigma, tinv32)
    stinv = single.tile([G, 1], F32)
    nc.scalar.sqrt(stinv, tinv32)

    Pcur = single.tile([G, G], F32, tag="P0")
    nc.vector.tensor_copy(Pcur, I32)
    for _ in range(T_ITERS):
        psa = psS.tile([G, G], F32, tag="sps")
        nc.tensor.matmul(psa, lhsT=Pcur, rhs=Pcur, start=True, stop=True)
        asb = ns.tile([G, G], F32, tag="nsA")
        nc.vector.tensor_copy(asb, psa)
        psb_ = psS.tile([G, G], F32, tag="sps")
        nc.tensor.matmul(psb_, lhsT=asb, rhs=Pcur, start=True, stop=True)
        bsb_ = ns.tile([G, G], F32, tag="nsB")
        nc.vector.tensor_copy(bsb_, psb_)
        psc = psS.tile([G, G], F32, tag="sps")
        nc.tensor.matmul(psc, lhsT=bsb_, rhs=sigN, start=True, stop=True)
        chalf = ns.tile([G, G], F32, tag="nsC")
        nc.vector.tensor_scalar_mul(chalf, psc, 0.5)
        pn = ns.tile([G, G], F32, tag="nsP")
        nc.vector.tensor_scalar_mul(pn, Pcur, 1.5)
        nc.vector.tensor_sub(pn, pn, chalf)
        Pcur = pn

    wm = single.tile([G, G], F32)
    nc.vector.tensor_scalar_mul(wm, Pcur, stinv)

    # block-diagonal WM = diag(wm, wm, wm, wm)
    ps_wm = psT.tile([P, P], F32, tag="pt")
    for i in range(4):
        nc.tensor.matmul(
            ps_wm[G * i : G * i + G, G * i : G * i + G],
            lhsT=wm,
            rhs=I32,
            start=True,
            stop=True,
            tile_position=(0, G * i),
        )
    WM = single.tile([P, P], F32)
    nc.vector.memset(WM, 0.0)
    for i in range(4):
        nc.vector.tensor_copy(
            WM[G * i : G * i + G, G * i : G * i + G],
            ps_wm[G * i : G * i + G, G * i : G * i + G],
        )

    # per-partition affine: scale = weight, bias = bias - (wm @ mu) * weight
    ps_v = psS.tile([G, 1], F32, tag="sps")
    nc.tensor.matmul(ps_v, lhsT=wm, rhs=mu, start=True, stop=True)
    vsb = single.tile([G, 1], F32)
    nc.vector.tensor_copy(vsb, ps_v)
    ps_v128 = psS.tile([P, 1], F32, tag="sps")
    nc.tensor.matmul(ps_v128, lhsT=BD[0:G, :], rhs=vsb, start=True, stop=True)
    v128 = single.tile([P, 1], F32)
    nc.vector.tensor_copy(v128, ps_v128)
    badj = single.tile([P, 2], F32)
    for h in range(2):
        nc.vector.tensor_mul(badj[:, h : h + 1], v128, wsb[:, h : h + 1])
        nc.vector.tensor_sub(badj[:, h : h + 1], bsb[:, h : h + 1], badj[:, h : h + 1])

    # ---------------- pass 2: normalize (fp32) ----------------
    half = HW // 2  # 1568
    for s in range(SLABS):
        if s < RESIDENT:
            xt = xt_tiles[s]
        else:
            xt = xstream.tile([P, HW], F32, tag="xs")
            nc.sync.dma_start(xt, x[s])
        h = s % 2
        osb = outp.tile([P, HW], F32, tag="osb")
        for grp in range(GRPS):
            off = 512 * grp
            wd = min(512, HW - off)
            py = psY.tile([P, 512], F32, tag="py")
            nc.tensor.matmul(
                py[:, 0:wd], lhsT=WM, rhs=xt[:, off : off + wd], start=True, stop=True
            )
            if grp % 2 == 0:
                nc.scalar.activation(
                    out=osb[:, off : off + wd],
                    in_=py[:, 0:wd],
                    func=AF.Identity,
                    bias=badj[:, h : h + 1],
                    scale=wsb[:, h : h + 1],
                )
            else:
                nc.vector.tensor_scalar(
                    out=osb[:, off : off + wd],
                    in0=py[:, 0:wd],
                    scalar1=wsb[:, h : h + 1],
                    scalar2=badj[:, h : h + 1],
                    op0=OP.mult,
                    op1=OP.add,
                )
        nc.sync.dma_start(out[s, :, 0:half], osb[:, 0:half])
        nc.sync.dma_start(out[s, :, half:HW], osb[:, half:HW])


_BUILT = None


def _build():
    global _BUILT
    if _BUILT is not None:
        return _BUILT
    nc = bacc.Bacc(
        "TRN2",
        target_bir_lowering=False,
        debug=False,
        enable_asserts=False,
        num_devices=N_CORES,
    )
    x_d = nc.dram_tensor("x", [SLABS, P, HW], F32, kind="ExternalInput")
    w_d = nc.dram_tensor("w2", [2, P, 1], F32, kind="ExternalInput")
    b_d = nc.dram_tensor("b2", [2, P, 1], F32, kind="ExternalInput")
    i_d = nc.dram_tensor("i128", [P, P], F32, kind="ExternalInput")
    bd_d = nc.dram_tensor("bd128", [P, P], F32, kind="ExternalInput")
    o_d = nc.dram_tensor("out", [SLABS, P, HW], F32, kind="ExternalOutput")
    from contextlib import ExitStack

    with tile.TileContext(nc) as tc, ExitStack() as ctx:
        _emit(ctx, tc, x_d.ap(), w_d.ap(), b_d.ap(), i_d.ap(), bd_d.ap(), o_d.ap())
    nc.compile()
    _BUILT = nc
    return nc


def kernel(x, weight, bias, trace=False, tmpdir=None):
    x = np.ascontiguousarray(np.asarray(x, dtype=np.float32))
    weight = np.asarray(weight, dtype=np.float32)
    bias = np.asarray(bias, dtype=np.float32)
    assert x.shape == (N, C, H, W)

    nc = _build()

    w2 = np.ascontiguousarray(weight.reshape(2, P, 1))
    b2 = np.ascontiguousarray(bias.reshape(2, P, 1))
    i128 = np.eye(P, dtype=np.float32)
    idx = np.arange(P)
    bd128 = (idx[:, None] % G == idx[None, :] % G).astype(np.float32)

    xs = x.reshape(N_CORES, SLABS, P, HW)
    in_maps = [
        {"x": xs[c], "w2": w2, "b2": b2, "i128": i128, "bd128": bd128}
        for c in range(N_CORES)
    ]
    res = bass_utils.run_bass_kernel_spmd(
        nc, in_maps, core_ids=list(range(N_CORES)), trace=trace, tmpdir=tmpdir
    )
    out = np.concatenate(
        [r["out"].reshape(1, N // N_CORES, C, H, W) for r in res.results], axis=0
    ).reshape(N, C, H, W)
    if trace:
        return out, res
    return out
